# revision 8
# baseline (speedup 1.0000x reference)
"""BlackwellLinear Trainium2 kernel: 2:4 sparsity + int8 fake-quant + x @ w.T + bias.

Full inputs in, full output out. Data-parallel over tokens across 8 NeuronCores;
weight/bias replicated. All module math (sparsify, quantize, matmul, bias) runs
on device; the host only re-encodes layouts: x is transposed to fp16, and the
in_features axis of both x.T and w.T is permuted phase-major
(p <-> 4*(p%256) + p//256), so each group-of-4 (the 2:4 sparsity unit) spans
four k-tiles at the SAME partition/column coordinates. The sparsify+quantize
pipeline is then contiguous full-width elementwise ops and the quantized weight
is produced directly in [in_f, out_f] (lhsT) layout -- no on-device transposes.

Numerics (harness rel-err gate 2e-2; this kernel lands ~1e-3):
  s   = absmax * (1/qmax)                  (fp32)
  inv ~= 1/s = qmax/absmax                 (reciprocal + 1 Newton on absmax)
  q   = rne(w * inv) via the magic-constant trick; clip is a no-op because
        |w| <= absmax ==> |w * inv| <= qmax*(1+eps) < qmax + 0.5.
  y   = s * (x16 @ (q * mask).T) + bias    (scale folded into PSUM eviction)
x is a single fp16 plane (~2^-11 rounding -> ~3e-4 on y); q <= 127 is
fp16-exact; accumulation is fp32 in PSUM; y is evicted fp16 and widened on the
host. The 2:4 threshold compare MUST run in fp32: fp16 |w| creates rounding
ties where the 3rd-largest of a group equals the 2nd, so is_ge keeps 3
elements and injects full-magnitude spurious weights (measured 5e-2 rel err).

Ramp design (the PE can only start after absmax -> inv -> first quantized
k-tile): all 8 w DMAs are issued before any compute so neither HWDGE queue
stalls behind engine work; per-k-tile column maxes pipeline with DMA arrivals;
k-tiles are consumed evens-first so one threshold build unblocks the PE's
first 4-k-tile sweep while range 1's threshold builds behind it.
"""

import numpy as np

N_CORES = 8
P = 128
IN_F = 1024
OUT_F = 1024
TOKENS = 32768
TOK_PER_CORE = TOKENS // N_CORES  # 4096
K_TILES = IN_F // P  # 8
M_TILES = OUT_F // P  # 8
TB_TOK = 2048  # token block per x strip
N_TB = TOK_PER_CORE // TB_TOK  # 2
MM_N = 512  # matmul moving free dim (one PSUM bank of fp32)
TJ = TB_TOK // MM_N  # 4 matmuls per (mi, ki) stationary load

MAGIC = 12582912.0  # 1.5 * 2**23: (v + MAGIC) - MAGIC == RNE round for |v| <= 2**22

# evens first: range-0 threshold (k-tiles 0,2,4,6) unblocks 4 quant chains
KT_ORDER = (0, 2, 4, 6, 1, 3, 5, 7)

# phase-major permutation of the in_features axis: position p holds original
# feature 4*(p%256) + p//256, so k-tile kt covers phase kt//2 of group range
# (kt%2)*128..+128 and the four phases of a group share partition/column coords
_PERM = (4 * (np.arange(IN_F) % 256) + np.arange(IN_F) // 256).astype(np.int64)

_CACHE = {}


def _build(qmax: float):
    from contextlib import ExitStack

    import concourse.tile as tile
    import concourse.mybir as mybir
    from concourse import bacc, bass_isa

    f32 = mybir.dt.float32
    f16 = mybir.dt.float16
    Alu = mybir.AluOpType
    Act = mybir.ActivationFunctionType

    inv_qmax = float(np.float32(1.0) / np.float32(qmax))
    nqmaxf = -float(np.float32(qmax))

    nc = bacc.Bacc("TRN2", target_bir_lowering=False, debug=False)
    xth = nc.dram_tensor("xth", [IN_F, TOK_PER_CORE], f16, kind="ExternalInput").ap()
    # wp: w.T with permuted in_f rows = [in_f_perm, out_f], fp32
    wp = nc.dram_tensor("wp", [IN_F, OUT_F], f32, kind="ExternalInput").ap()
    bias = nc.dram_tensor("bias", [OUT_F], f32, kind="ExternalInput").ap()
    yt = nc.dram_tensor("yt", [OUT_F, TOK_PER_CORE], f16, kind="ExternalOutput").ap()

    with tile.TileContext(nc) as tc, ExitStack() as ctx:
        const = ctx.enter_context(tc.tile_pool(name="const", bufs=1))
        wnat_p = ctx.enter_context(tc.tile_pool(name="wnat", bufs=8))
        abs_p = ctx.enter_context(tc.tile_pool(name="absp", bufs=8))
        tree_p = ctx.enter_context(tc.tile_pool(name="tree", bufs=4))
        thr_p = ctx.enter_context(tc.tile_pool(name="thr", bufs=2))
        t12_p = ctx.enter_context(tc.tile_pool(name="t12", bufs=1))
        mins_p = ctx.enter_context(tc.tile_pool(name="mins", bufs=2))
        mask_p = ctx.enter_context(tc.tile_pool(name="mask", bufs=3))
        qtmp_p = ctx.enter_context(tc.tile_pool(name="qtmp", bufs=2))
        wqt_p = ctx.enter_context(tc.tile_pool(name="wqt", bufs=8))
        sc_p = ctx.enter_context(tc.tile_pool(name="sc", bufs=1))
        x_p = ctx.enter_context(tc.tile_pool(name="x", bufs=11))
        y_p = ctx.enter_context(tc.tile_pool(name="y", bufs=3))
        psum_mm = ctx.enter_context(tc.tile_pool(name="psmm", bufs=8, space="PSUM"))

        def vts(out, in0, s1, op0, s2=None, op1=None):
            kw = {"op1": op1} if op1 is not None else {}
            nc.vector.tensor_scalar(
                out=out, in0=in0, scalar1=s1, scalar2=s2, op0=op0, **kw
            )

        def vtt(out, in0, in1, op):
            nc.vector.tensor_tensor(out=out, in0=in0, in1=in1, op=op)

        # ---- all weight DMAs first, evens in each queue's first slots ----
        wk = [None] * K_TILES
        for eng, kts in ((nc.sync, (0, 2, 1, 3)), (nc.scalar, (4, 6, 5, 7))):
            for kt in kts:
                wt = wnat_p.tile([P, OUT_F], f32, tag="wnat", name=f"wnat{kt}")
                eng.dma_start(wt[:], wp[kt * P : (kt + 1) * P, :])
                wk[kt] = wt
        magic_t = sc_p.tile([P, 1], f32, tag="magic")
        nc.gpsimd.memset(magic_t[:], MAGIC)

        # ---- |w| (ACT), in expected arrival order ----
        ak = [None] * K_TILES
        for kt in (0, 4, 2, 6, 1, 5, 3, 7):
            a = abs_p.tile([P, OUT_F], f32, tag="abs", name=f"abs{kt}")
            nc.scalar.activation(a[:], wk[kt][:], Act.Abs)
            ak[kt] = a

        # pair maxes fused with their column max (feeds global absmax):
        # tA_r = max(|w_r|,|w_r+2|), tB_r = max(|w_r+4|,|w_r+6|)
        accs = {}

        def pair_max(r, half):
            lo = r + 4 * half
            t = tree_p.tile([P, OUT_F], f32, tag="tAB", name=f"tAB_{r}_{half}")
            vtt(t[:], ak[lo][:], ak[lo + 2][:], Alu.max)
            acc = sc_p.tile([P, 1], f32, tag=f"acc_{r}_{half}")
            nc.vector.tensor_reduce(
                out=acc[:], in_=t[:], axis=mybir.AxisListType.X, op=Alu.max
            )
            accs[(r, half)] = acc
            return t

        def finish_thr(r, tA, tB):
            t1 = t12_p.tile([P, OUT_F], f32, tag="t1", name=f"t1_{r}")
            vtt(t1[:], tA[:], tB[:], Alu.min)
            mA = mins_p.tile([P, OUT_F], f32, tag="mins", name=f"mA_{r}")
            vtt(mA[:], ak[r][:], ak[r + 2][:], Alu.min)
            mB = mins_p.tile([P, OUT_F], f32, tag="mins", name=f"mB_{r}")
            vtt(mB[:], ak[r + 4][:], ak[r + 6][:], Alu.min)
            t2 = t12_p.tile([P, OUT_F], f32, tag="t2", name=f"t2_{r}")
            vtt(t2[:], mA[:], mB[:], Alu.max)
            tr = thr_p.tile([P, OUT_F], f32, tag="thr", name=f"thr_{r}")
            vtt(tr[:], t1[:], t2[:], Alu.max)
            return tr

        def mk_mask(kt, tr):
            m = mask_p.tile([P, OUT_F], f16, tag="mask", name=f"m_{kt}")
            vtt(m[:], ak[kt][:], tr[:], Alu.is_ge)
            return m

        # range-0 threshold + kt0 mask ahead of everything downstream
        tA0 = pair_max(0, 0)
        tB0 = pair_max(0, 1)
        thr0 = finish_thr(0, tA0, tB0)
        m0 = mk_mask(0, thr0)
        # absmax path: range-1 pair maxes (their |w| land last), combine,
        # cross-partition reduce, inv ~= qmax/absmax (recip + 1 Newton)
        tA1 = pair_max(1, 0)
        tB1 = pair_max(1, 1)
        am0 = sc_p.tile([P, 1], f32, tag="am0")
        am1 = sc_p.tile([P, 1], f32, tag="am1")
        amc = sc_p.tile([P, 1], f32, tag="amc")
        vtt(am0[:], accs[(0, 0)][:], accs[(0, 1)][:], Alu.max)
        vtt(am1[:], accs[(1, 0)][:], accs[(1, 1)][:], Alu.max)
        vtt(amc[:], am0[:], am1[:], Alu.max)
        am = sc_p.tile([P, 1], f32, tag="am")
        nc.gpsimd.partition_all_reduce(
            am[:], amc[:], channels=P, reduce_op=bass_isa.ReduceOp.max
        )
        r0 = sc_p.tile([P, 1], f32, tag="r0")
        nc.vector.reciprocal(r0[:], am[:])
        p1 = sc_p.tile([P, 1], f32, tag="p1")
        e1 = sc_p.tile([P, 1], f32, tag="e1")
        r1 = sc_p.tile([P, 1], f32, tag="r1")
        inv_t = sc_p.tile([P, 1], f32, tag="inv")
        vtt(p1[:], am[:], r0[:], Alu.mult)
        vts(e1[:], p1[:], 2.0, Alu.subtract)  # p1 - 2 = -(2 - p1)
        vtt(r1[:], r0[:], e1[:], Alu.mult)  # -r0*(2 - p1) ~= -1/absmax
        vts(inv_t[:], r1[:], nqmaxf, Alu.mult)  # qmax/absmax
        s_t = sc_p.tile([P, 1], f32, tag="s")
        vts(s_t[:], am[:], inv_qmax, Alu.mult)

        # ---- per k-tile: q0 = rne(w*inv)+MAGIC (ACT), mask (DVE f32
        # compare -> f16), q16 = (q0 - MAGIC)*m downcast fp16 (DVE).
        # Delivery order matches the PE's tb0 sweep; range-1's threshold
        # builds right after kt0 ships so the odd k-tiles arrive in time. ----
        masks = {0: m0}
        wqt_by_kt = {}

        def quant(kt):
            q0 = qtmp_p.tile([P, OUT_F], f32, tag="q0", name=f"q0_{kt}")
            nc.scalar.activation(
                q0[:], wk[kt][:], Act.Identity, bias=magic_t[:], scale=inv_t[:]
            )
            q16 = wqt_p.tile([P, OUT_F], f16, tag="q16", name=f"q16_{kt}")
            nc.vector.scalar_tensor_tensor(
                out=q16[:], in0=q0[:], scalar=-MAGIC, in1=masks[kt][:],
                op0=Alu.add, op1=Alu.mult,
            )
            wqt_by_kt[kt] = q16

        quant(0)
        thr1 = finish_thr(1, tA1, tB1)
        for kt in (2, 4, 6):
            masks[kt] = mk_mask(kt, thr0)
            quant(kt)
        for kt in (1, 3, 5, 7):
            masks[kt] = mk_mask(kt, thr1)
            quant(kt)
        wqt = [wqt_by_kt[kt] for kt in range(K_TILES)]

        # ---- bias slices (needed only by first eviction; issue late) ----
        bias_t = []
        for mi in range(M_TILES):
            bt = const.tile([P, 1], f32, tag=f"bias{mi}")
            nc.scalar.dma_start(bt[:, 0:1], bias[mi * P : (mi + 1) * P].unsqueeze(1))
            bias_t.append(bt)

        # ---- main matmul: yt[m, t] = sum_k wqt[k,m].T @ x16[k,t] ----
        for tb in range(N_TB):
            xh = {}
            for i, ki in enumerate(KT_ORDER):
                sl_p = slice(ki * P, (ki + 1) * P)
                sl_t = slice(tb * TB_TOK, (tb + 1) * TB_TOK)
                xht = x_p.tile([P, TB_TOK], f16, tag="xh", name=f"xh{tb}_{ki}")
                (nc.sync if i % 2 == 0 else nc.scalar).dma_start(xht[:], xth[sl_p, sl_t])
                xh[ki] = xht

            last_tb = tb == N_TB - 1

            def evict(mi, ps_tj):
                # 4 psum banks -> one [P, TB_TOK] fp16 tile, engines split by
                # tj parity (ACT/DVE hit different banks in parallel). On the
                # last token block, DMA per bank-pair so the drain overlaps
                # the final evictions instead of serializing after them.
                ysb = y_p.tile([P, TB_TOK], f16, tag="ysb", name=f"y{tb}_{mi}")
                for tj in range(TJ):
                    dst = ysb[:, tj * MM_N : (tj + 1) * MM_N]
                    if tj % 2 == 0:
                        nc.scalar.activation(
                            dst,
                            ps_tj[tj][:],
                            Act.Identity,
                            bias=bias_t[mi][:],
                            scale=s_t[:],
                        )
                    else:
                        nc.vector.tensor_scalar(
                            out=dst,
                            in0=ps_tj[tj][:],
                            scalar1=s_t[:],
                            scalar2=bias_t[mi][:],
                            op0=Alu.mult,
                            op1=Alu.add,
                        )
                    if last_tb and tj % 2 == 1:
                        tcol = tb * TB_TOK + (tj - 1) * MM_N
                        (nc.sync if (mi + tj) % 2 == 0 else nc.scalar).dma_start(
                            yt[mi * P : (mi + 1) * P, tcol : tcol + 2 * MM_N],
                            ysb[:, (tj - 1) * MM_N : (tj + 1) * MM_N],
                        )
                if not last_tb:
                    tcol = tb * TB_TOK
                    (nc.sync if mi % 2 == 0 else nc.scalar).dma_start(
                        yt[mi * P : (mi + 1) * P, tcol : tcol + TB_TOK], ysb[:]
                    )

            if tb == 0:
                # k-outer sweep: PE starts as soon as the first quantized
                # k-tile lands, consuming k-tiles at the prep pipeline's pace
                for mh in range(4):
                    ps = {
                        (ml, tj): psum_mm.tile(
                            [P, MM_N], f32, tag="ps", name=f"ps0_{mh}_{ml}_{tj}"
                        )
                        for ml in range(2)
                        for tj in range(TJ)
                    }
                    for i, ki in enumerate(KT_ORDER):
                        for ml in range(2):
                            mi = mh * 2 + ml
                            lhsT = wqt[ki][:, mi * P : (mi + 1) * P]
                            for tj in range(TJ):
                                nc.tensor.matmul(
                                    ps[ml, tj][:],
                                    lhsT,
                                    xh[ki][:, tj * MM_N : (tj + 1) * MM_N],
                                    start=(i == 0),
                                    stop=(i == K_TILES - 1),
                                )
                    for ml in range(2):
                        mi = mh * 2 + ml
                        evict(mi, [ps[ml, tj] for tj in range(TJ)])
            else:
                for mi in range(M_TILES):
                    ps = [
                        psum_mm.tile(
                            [P, MM_N], f32, tag="ps", name=f"ps{tb}_{mi}_{tj}"
                        )
                        for tj in range(TJ)
                    ]
                    for i, ki in enumerate(KT_ORDER):
                        lhsT = wqt[ki][:, mi * P : (mi + 1) * P]
                        for tj in range(TJ):
                            nc.tensor.matmul(
                                ps[tj][:],
                                lhsT,
                                xh[ki][:, tj * MM_N : (tj + 1) * MM_N],
                                start=(i == 0),
                                stop=(i == K_TILES - 1),
                            )
                    evict(mi, ps)

    nc.compile()
    return nc


def _get(qmax: float):
    key = qmax
    if key not in _CACHE:
        _CACHE[key] = _build(qmax)
    return _CACHE[key]


def host_prep(x, weight):
    """Host-side input re-encoding: transpose, phase-major permute the in_f
    axis, fp16 downcast of x. Pure layout/encoding; no module math."""
    xt = np.ascontiguousarray(x.T)[_PERM]  # [IN_F perm, TOKENS]
    xth = xt.astype(np.float16)
    wp = np.ascontiguousarray(weight.T[_PERM])  # [IN_F perm, OUT_F]
    return xth, wp


LAST_EXEC_NS = None


def kernel(x, weight, bias, precision, _trace_dir=None):
    global LAST_EXEC_NS
    from concourse.bass_utils import run_bass_kernel_spmd

    x = np.asarray(x, dtype=np.float32)
    weight = np.asarray(weight, dtype=np.float32)
    bias = np.asarray(bias, dtype=np.float32)
    prec = int(np.asarray(precision))
    qmax = float(2 ** (prec - 1) - 1)

    nc = _get(qmax)

    xth, wp = host_prep(x, weight)
    in_maps = [
        {
            "xth": np.ascontiguousarray(
                xth[:, c * TOK_PER_CORE : (c + 1) * TOK_PER_CORE]
            ),
            "wp": wp,
            "bias": bias,
        }
        for c in range(N_CORES)
    ]
    kw = {}
    if _trace_dir is not None:
        kw = {"trace": True, "tmpdir": _trace_dir}
    res = run_bass_kernel_spmd(nc, in_maps, list(range(N_CORES)), **kw)
    LAST_EXEC_NS = res.exec_time_ns
    yt = np.concatenate([res.results[c]["yt"] for c in range(N_CORES)], axis=1)
    return np.ascontiguousarray(yt.T.astype(np.float32))
